# revision 7
# baseline (speedup 1.0000x reference)
"""Trainium2 Bass kernel for nn_Mixer: two rounds of InstanceNorm -> 1x1 conv -> ReLU.

Reference computation (per sample b):
    h   = relu(W1 @ IN(x_b) + b1)      x_b: [256, 16384]
    out = relu(W2 @ IN(h)   + b2)

Strategy:
  * Data-parallel over batch: 16 samples / 8 cores = 2 samples per core.
  * InstanceNorm is folded into the conv: IN(x) = (x - mu) * s with s =
    rsqrt(var + eps), so W @ IN(x) = (W diag(s)) @ x - (W diag(s)) mu.
    Only the tiny [256, 256] weights are rescaled per sample; the big
    activations are never normalized elementwise.
  * Stats: bn_stats/bn_aggr on the vector engine for x; for h the conv1
    epilogue (scalar engine activation) accumulates sum(h) for free and a
    single fused square-accumulate vector pass yields sum(h^2).
  * Convs run on the tensor engine as [K=256] x [M=256] x [N=16384]
    matmuls in float32r (full PE rate), fp32 accumulation in PSUM.
  * One big SBUF region (16 tiles of [128, 2048]) is chained
    x(s) -> h(s) -> x(s+1) -> h(s+1) via tile-pool tag reuse, so the
    next sample's DMA-in overlaps the current sample's conv2.
"""

import sys

for _p in ("/opt/trn_rl_repo",):
    if _p not in sys.path:
        sys.path.append(_p)

from contextlib import ExitStack

import numpy as np

import bass_rust
import concourse.bass as bass
import concourse.tile as tile
from concourse import mybir
from concourse.bass_utils import run_bass_kernel_spmd
from concourse.vector_clock import ScopedClock

# Problem shape (hardcoded per contract)
B, C, H, W = 16, 256, 128, 128
HW = H * W                      # 16384
NCORES = 8
SPB = B // NCORES               # samples per core = 2
P = 128                         # partitions
KT = C // P                     # 2 contraction tiles
MT = C // P                     # 2 output-channel tiles
NGRP = 8                        # column groups per sample
GRP = HW // NGRP                # 2048 columns per group
MMN = 512                       # matmul free dim (one PSUM bank of fp32)
NCHUNK = GRP // MMN             # 4 matmuls per group per (m, k)
EPS = 1e-5
F32 = mybir.dt.float32

# Dtype used for the big conv matmuls. float32r runs the PE at full rate
# (4x fp32) with slightly reduced internal precision; flip to mybir.dt.float32
# if the accuracy budget ever demands exact fp32 matmuls.
MM_DT = mybir.dt.float32r


def _patched_drain_and_barrier(self, tick_clock, wait_clock):
    # The pinned walrus build rejects instructions carrying more than one
    # sync-wait command ("Too many sync wait commands", CoreV3GenImpl
    # setupSyncWait). Tile's stock epilogue hangs every final semaphore wait
    # on the single SP Drain. Collect those waits, strip them off the drain,
    # and re-emit each as its own single-wait instruction on the vector queue.
    drain_inst = self.nc.sync.drain()
    wait_clock.add_sem_waits(
        drain_inst.ins, ScopedClock({None: tick_clock.global_clock})
    )
    waits = list(drain_inst.ins.sync_info.on_wait)
    drain_inst.ins.sync_info = bass_rust.SyncInfo(on_wait=[], on_update=[])
    assert self.sems is not None
    by_name = {h.name: h for h in self.sems.allocated().values()}
    for w in waits:
        h = by_name.get(w.ant_name)
        assert h is not None, (w.ant_name, sorted(by_name))
        self.nc.vector.wait_ge(h, w.wait_value)
    self.nc.all_engine_barrier()
    popped = self.nc._tile_sem_poison_stack.pop()
    assert popped is self._sem_poison
    self.nc.clear_and_free_semaphores(list(self.sems.allocated().values()))
    self.nc.all_engine_barrier()


tile.TileContext._drain_and_barrier = _patched_drain_and_barrier

_MAX_WAITS = 1  # this walrus build rejects >1 sync-wait command per instruction


def _split_multi_waits(nc):
    """Hoist excess semaphore waits onto standalone EventSemaphore
    instructions (same engine, inserted immediately before), because the
    pinned walrus rejects instructions carrying more than one sync wait."""
    counter = [0]
    for fn in nc.m.functions:
        for bb in fn.blocks:
            insns = bb.instructions
            if not any(
                ins.sync_info is not None
                and ins.sync_info.on_wait
                and len(ins.sync_info.on_wait) > _MAX_WAITS
                for ins in insns
            ):
                continue
            out = []
            for ins in insns:
                si = ins.sync_info
                waits = list(si.on_wait) if si is not None and si.on_wait else []
                if len(waits) > _MAX_WAITS:
                    for w in waits[: -_MAX_WAITS]:
                        counter[0] += 1
                        ev = mybir.InstEventSemaphore(
                            name=f"I-waitsplit-{counter[0]}", ins=[], outs=[]
                        )
                        ev.engine = ins.engine
                        ev.sync_info = bass_rust.SyncInfo(
                            on_wait=[w], on_update=[]
                        )
                        nc.register_instruction(ev)
                        out.append(ev)
                    ins.sync_info = bass_rust.SyncInfo(
                        on_wait=waits[-_MAX_WAITS:],
                        on_update=list(si.on_update) if si.on_update else [],
                    )
                out.append(ins)
            bb.instructions = out


def _emit_sample(nc, ctx, tc, pools, aps, si):
    """Emit the full pipeline for one sample."""
    chain = pools["chain"]
    psum = pools["psum"]
    stage = pools["stage"]
    sq = pools["sq"]
    stats = pools["stats"]
    wfold = pools["wfold"]

    x_r = aps["x"]          # [SPB, KT, P, HW]
    out_r = aps["out"]      # [SPB, MT, P, HW]
    w1t_sb = aps["w1t_sb"]  # list of KT tiles [P, C]  (w1.T, rows = in-chan)
    w2t_sb = aps["w2t_sb"]
    b1_sb = aps["b1_sb"]    # [P, MT]
    b2_sb = aps["b2_sb"]
    eps_sb = aps["eps_sb"]  # [P, 1]

    # ---- Stage A: DMA x in, per-chunk bn_stats --------------------------
    xtiles = {}
    xstat = []
    for k in range(KT):
        st_k = stats.tile([P, NGRP * NCHUNK, 6], F32, tag=f"xstat{k}", name=f"xstat{k}")
        xstat.append(st_k)
        for g in range(NGRP):
            xt = chain.tile([P, GRP], MM_DT, tag=f"chain_{k}_{g}", name=f"x_{k}_{g}")
            xtiles[(k, g)] = xt
            nc.sync.dma_start(out=xt, in_=x_r[si, k, :, g * GRP:(g + 1) * GRP].bitcast(MM_DT))
            for cch in range(NCHUNK):
                nc.vector.bn_stats(
                    out=st_k[:, g * NCHUNK + cch, :],
                    in_=xt[:, cch * MMN:(cch + 1) * MMN].bitcast(F32),
                )

    # ---- Stage B: aggregate x stats, fold conv1 weights -----------------
    w1p = []
    mu1 = []
    for k in range(KT):
        mv = stats.tile([P, 2], F32, tag=f"xmv{k}", name=f"xmv{k}")
        nc.vector.bn_aggr(out=mv, in_=xstat[k])
        s1 = stats.tile([P, 1], F32, tag=f"s1_{k}", name=f"s1_{k}")
        nc.scalar.activation(
            out=s1, in_=mv[:, 1:2],
            func=mybir.ActivationFunctionType.Sqrt, bias=eps_sb,
        )
        nc.vector.reciprocal(out=s1, in_=s1)
        wp = wfold.tile([P, C], MM_DT, tag=f"w1p{k}", name=f"w1p{k}")
        nc.vector.tensor_scalar_mul(out=wp, in0=w1t_sb[k], scalar1=s1)
        w1p.append(wp)
        mu_r = stats.tile([P, 2], MM_DT, tag=f"mu1r{k}", name=f"mu1r{k}")
        nc.vector.tensor_copy(out=mu_r[:, 0:1], in_=mv[:, 0:1])
        nc.vector.tensor_copy(out=mu_r[:, 1:2], in_=mv[:, 0:1])
        mu1.append(mu_r)

    bias1 = []
    for m in range(MT):
        pb = psum.tile([P, GRP], F32, tag="ps", name="ps")
        for k in range(KT):
            nc.tensor.matmul(
                pb[:, 0:2],
                lhsT=w1p[k][:, m * P:(m + 1) * P],
                rhs=mu1[k],
                start=(k == 0), stop=(k == KT - 1),
            )
        bm = stats.tile([P, 1], F32, tag=f"bias1_{m}", name=f"bias1_{m}")
        nc.vector.tensor_tensor(
            out=bm, in0=b1_sb[:, m:m + 1], in1=pb[:, 0:1],
            op=mybir.AluOpType.subtract,
        )
        bias1.append(bm)

    # ---- Stage C: conv1 + relu epilogue + h partial stats ---------------
    htiles = {}
    hsum = []
    hsq = []
    for m in range(MT):
        hsum.append(stats.tile([P, NGRP], F32, tag=f"hsum{m}", name=f"hsum{m}"))
        hsq.append(stats.tile([P, NGRP], F32, tag=f"hsq{m}", name=f"hsq{m}"))
    for g in range(NGRP):
        for m in range(MT):
            ps = psum.tile([P, GRP], F32, tag="ps", name="ps")
            for k in range(KT):
                lhs = w1p[k][:, m * P:(m + 1) * P]
                xt = xtiles[(k, g)]
                for cch in range(NCHUNK):
                    nc.tensor.matmul(
                        ps[:, cch * MMN:(cch + 1) * MMN],
                        lhsT=lhs,
                        rhs=xt[:, cch * MMN:(cch + 1) * MMN],
                        start=(k == 0), stop=(k == KT - 1),
                    )
            ht = chain.tile([P, GRP], MM_DT, tag=f"chain_{m}_{g}", name=f"h_{m}_{g}")
            htiles[(m, g)] = ht
            nc.scalar.activation(
                out=ht, in_=ps, func=mybir.ActivationFunctionType.Relu,
                bias=bias1[m], accum_out=hsum[m][:, g:g + 1],
            )
            sqt = sq.tile([P, GRP], F32, tag="sq", name="sqt")
            nc.vector.scalar_tensor_tensor(
                out=sqt, in0=ht.bitcast(F32), scalar=1.0, in1=ht.bitcast(F32),
                op0=mybir.AluOpType.mult, op1=mybir.AluOpType.mult,
                accum_out=hsq[m][:, g:g + 1],
            )

    # ---- Stage D: aggregate h stats, fold conv2 weights -----------------
    w2p = []
    mu2 = []
    for m in range(MT):
        hm = stats.tile([P, 1], F32, tag=f"hmean{m}", name=f"hmean{m}")
        nc.vector.reduce_sum(out=hm, in_=hsum[m], axis=mybir.AxisListType.X)
        nc.scalar.mul(out=hm, in_=hm, mul=1.0 / HW)
        hq = stats.tile([P, 1], F32, tag=f"hmsq{m}", name=f"hmsq{m}")
        nc.vector.reduce_sum(out=hq, in_=hsq[m], axis=mybir.AxisListType.X)
        nc.scalar.mul(out=hq, in_=hq, mul=1.0 / HW)
        # var = E[h^2] - mean^2 ; s2 = rsqrt(var + eps)
        msq = stats.tile([P, 1], F32, tag=f"hmsq2{m}", name=f"hmsq2{m}")
        nc.vector.tensor_mul(out=msq, in0=hm, in1=hm)
        s2 = stats.tile([P, 1], F32, tag=f"s2_{m}", name=f"s2_{m}")
        nc.vector.tensor_tensor(
            out=s2, in0=hq, in1=msq, op=mybir.AluOpType.subtract,
        )
        nc.scalar.activation(
            out=s2, in_=s2,
            func=mybir.ActivationFunctionType.Sqrt, bias=eps_sb,
        )
        nc.vector.reciprocal(out=s2, in_=s2)
        wp = wfold.tile([P, C], MM_DT, tag=f"w2p{m}", name=f"w2p{m}")
        nc.vector.tensor_scalar_mul(out=wp, in0=w2t_sb[m], scalar1=s2)
        w2p.append(wp)
        mu_r = stats.tile([P, 2], MM_DT, tag=f"mu2r{m}", name=f"mu2r{m}")
        nc.vector.tensor_copy(out=mu_r[:, 0:1], in_=hm)
        nc.vector.tensor_copy(out=mu_r[:, 1:2], in_=hm)
        mu2.append(mu_r)

    bias2 = []
    for mo in range(MT):
        pb = psum.tile([P, GRP], F32, tag="ps", name="ps")
        for m in range(MT):
            nc.tensor.matmul(
                pb[:, 0:2],
                lhsT=w2p[m][:, mo * P:(mo + 1) * P],
                rhs=mu2[m],
                start=(m == 0), stop=(m == MT - 1),
            )
        bm = stats.tile([P, 1], F32, tag=f"bias2_{mo}", name=f"bias2_{mo}")
        nc.vector.tensor_tensor(
            out=bm, in0=b2_sb[:, mo:mo + 1], in1=pb[:, 0:1],
            op=mybir.AluOpType.subtract,
        )
        bias2.append(bm)

    # ---- Stage E: conv2 + relu epilogue + DMA out -----------------------
    for g in range(NGRP):
        for mo in range(MT):
            ps = psum.tile([P, GRP], F32, tag="ps", name="ps")
            for m in range(MT):
                lhs = w2p[m][:, mo * P:(mo + 1) * P]
                ht = htiles[(m, g)]
                for cch in range(NCHUNK):
                    nc.tensor.matmul(
                        ps[:, cch * MMN:(cch + 1) * MMN],
                        lhsT=lhs,
                        rhs=ht[:, cch * MMN:(cch + 1) * MMN],
                        start=(m == 0), stop=(m == MT - 1),
                    )
            og = stage.tile([P, GRP], F32, tag="og", name="og")
            nc.scalar.activation(
                out=og, in_=ps, func=mybir.ActivationFunctionType.Relu,
                bias=bias2[mo],
            )
            nc.sync.dma_start(
                out=out_r[si, mo, :, g * GRP:(g + 1) * GRP], in_=og,
            )


def build_program():
    nc = bass.Bass()
    x = nc.dram_tensor("x", [SPB, C, HW], F32, kind="ExternalInput")
    w1t = nc.dram_tensor("w1t", [C, C], F32, kind="ExternalInput")
    b1 = nc.dram_tensor("b1", [MT, P], F32, kind="ExternalInput")
    w2t = nc.dram_tensor("w2t", [C, C], F32, kind="ExternalInput")
    b2 = nc.dram_tensor("b2", [MT, P], F32, kind="ExternalInput")
    out = nc.dram_tensor("out", [SPB, C, HW], F32, kind="ExternalOutput")

    with ExitStack() as ctx:
        tc = ctx.enter_context(tile.TileContext(nc))
        pools = {
            # x(s) -> h(s) -> x(s+1) chain: 16 x [128, 2048] fp32 = 128KB/part
            "chain": ctx.enter_context(tc.tile_pool(name="chain", bufs=1)),
            "psum": ctx.enter_context(
                tc.tile_pool(name="psum", bufs=2, space="PSUM")
            ),
            "stage": ctx.enter_context(tc.tile_pool(name="stage", bufs=3)),
            "sq": ctx.enter_context(tc.tile_pool(name="sq", bufs=2)),
            "stats": ctx.enter_context(tc.tile_pool(name="stats", bufs=2)),
            "wfold": ctx.enter_context(tc.tile_pool(name="wfold", bufs=2)),
            "singles": ctx.enter_context(tc.tile_pool(name="singles", bufs=1)),
        }
        singles = pools["singles"]

        aps = {
            "x": x.ap().rearrange("s (k p) n -> s k p n", p=P),
            "out": out.ap().rearrange("s (m p) n -> s m p n", p=P),
        }
        # weights (already transposed host-side: rows = input channel)
        w1t_r = w1t.ap().rearrange("(k p) o -> k p o", p=P)
        w2t_r = w2t.ap().rearrange("(k p) o -> k p o", p=P)
        aps["w1t_sb"] = []
        aps["w2t_sb"] = []
        for k in range(KT):
            t1 = singles.tile([P, C], F32, tag=f"w1t{k}", name=f"w1t{k}")
            nc.sync.dma_start(out=t1, in_=w1t_r[k])
            aps["w1t_sb"].append(t1)
            t2 = singles.tile([P, C], F32, tag=f"w2t{k}", name=f"w2t{k}")
            nc.sync.dma_start(out=t2, in_=w2t_r[k])
            aps["w2t_sb"].append(t2)
        b1_sb = singles.tile([P, MT], F32, tag="b1", name="b1sb")
        nc.sync.dma_start(out=b1_sb, in_=b1.ap().rearrange("m p -> p m"))
        aps["b1_sb"] = b1_sb
        b2_sb = singles.tile([P, MT], F32, tag="b2", name="b2sb")
        nc.sync.dma_start(out=b2_sb, in_=b2.ap().rearrange("m p -> p m"))
        aps["b2_sb"] = b2_sb
        eps_sb = singles.tile([P, 1], F32, tag="eps", name="epssb")
        nc.vector.memset(eps_sb, EPS)
        aps["eps_sb"] = eps_sb

        for si in range(SPB):
            _emit_sample(nc, ctx, tc, pools, aps, si)

    _split_multi_waits(nc)
    return nc


_CACHED_NC = None


def _get_program():
    global _CACHED_NC
    if _CACHED_NC is None:
        _CACHED_NC = build_program()
    return _CACHED_NC


def _make_in_maps(x, w1, b1, w2, b2):
    xs = np.ascontiguousarray(
        x.reshape(NCORES, SPB, C, HW).astype(np.float32, copy=False)
    )
    w1t = np.ascontiguousarray(w1.T.astype(np.float32, copy=False))
    w2t = np.ascontiguousarray(w2.T.astype(np.float32, copy=False))
    b1r = np.ascontiguousarray(b1.reshape(MT, P).astype(np.float32, copy=False))
    b2r = np.ascontiguousarray(b2.reshape(MT, P).astype(np.float32, copy=False))
    return [
        {"x": xs[i], "w1t": w1t, "b1": b1r, "w2t": w2t, "b2": b2r}
        for i in range(NCORES)
    ]


def kernel(x, w1, b1, w2, b2, _trace=False):
    nc = _get_program()
    in_maps = _make_in_maps(x, w1, b1, w2, b2)
    res = run_bass_kernel_spmd(nc, in_maps, list(range(NCORES)), trace=_trace)
    out = np.concatenate([r["out"][None] for r in res.results], axis=0)
    out = out.reshape(B, C, H, W).astype(np.float32, copy=False)
    if _trace:
        return out, res
    return out
